# revision 10
# baseline (speedup 1.0000x reference)
"""BlockwiseDense Trainium2 kernel (8 NeuronCores, sharded over out_blocks).

Math (per reference):
    w = rram_quantize(relu(cores))          # snap to 256 log-spaced levels
    y[b,i,j,k] = sum_l w[i,j,k,l] * x[b,j,l]

The quantizer index s(w) = MULT*ln((A-w)/B) + C0 is approximated by the
baseline's quadratic s ~= A2M*w^2 + P1*w + P0 (~0.06% level flips).
The round-and-clamp sat_u8(rne(s)) is done with a magic-number trick:
    m = f16(max(s + P0 + 1024, 1024))
fp16 has ulp=1 on [1024,2048), so the cast itself performs RNE to an
integer grid, and the max() reproduces the relu/low-side clamp (negative
weights land on level 0 = g_min exactly as relu+quantize would).  Exp's
bias absorbs the -1024*ln_r offset:  e = Exp(ln_r*m - 1024*ln_r).

Engine split: ten "V" j-blocks run the chain on DVE in fp16 (2x rate):
ts gg -> tt hh -> ts m; six "G" j-blocks run it on GpSimd via fp32
intermediates (GpSimd is ~2x slower on all-fp16 ops but full speed when
any operand is fp32).  ACT runs Exp for all blocks plus the PSUM->SBUF
evict for G blocks (Identity with per-partition bias = A*s2); DVE evicts
V blocks reading the s2 column straight out of PSUM as the tensor_scalar
vector operand.  s2 = sum_l x is accumulated on the PE by a ones-matmul
whose "ones" are preset to -A/B so no separate scale pass is needed.

fp16 matmuls (512-wide moving operand, half PE rate) accumulate in fp32
PSUM over the two 128-row halves of l.  All weight-granule DMAs are
issued upfront on the sync ring so the 16 HW DMA queues stay
descriptor-fed; x rides the gpsimd ring in two chunks; y stores follow
on the sync ring.  Core c takes out_blocks {2c, 2c+1}.
"""

import numpy as np

import concourse.bacc as bacc
import concourse.mybir as mybir
from concourse.tile import TileContext
from concourse.bass_utils import run_bass_kernel_spmd

BATCH = 128
IN_BLOCKS = 16
OUT_BLOCKS = 16
NB = 256
N_CORES = 8
I_PER_CORE = OUT_BLOCKS // N_CORES  # 2
IK = I_PER_CORE * NB  # 512

TAU, G_INF, G_MIN, L = 0.75, 2.0, 0.001, 256
B_SCALE = (G_INF - G_MIN) / (1.0 - float(np.exp(-TAU)))
A_OFF = G_MIN + B_SCALE
MULT = -(L - 1) / TAU
LN_R = -TAU / (L - 1)

# quadratic fit s(w) ~= A2M*w^2 + P1*w + P0 (baseline constants)
C0 = 0.5 - float(np.log((1 + np.exp(LN_R)) / 2) / LN_R)
_c1 = -C0 / 340.0
_g2 = -340.0 - 170.0 * _c1
C1W = -(G_MIN + B_SCALE * _c1)
A2M = 170.0 / (B_SCALE * B_SCALE)
A2B = -(170.0 * G_MIN / B_SCALE + _g2) / B_SCALE
P1 = A2B + C1W * A2M
P0 = C1W * A2B

MAGIC = 1024.0
# exact fp32 cancellation of ln_r*1024 so n=0 gives e=1.0 exactly
EXP_BIAS = float(-(np.float32(LN_R) * np.float32(MAGIC)))

F32 = mybir.dt.float32
F16 = mybir.dt.float16
U8 = mybir.dt.uint8

# granules: (j-list, engine) - "v" chain on DVE fp16, "g" on GpSimd fp32.
GSPEC = [
    ([0], "g"),
    ([1], "g"),
    ([2, 3], "v"),
    ([4], "g"),
    ([5], "g"),
    ([6, 7], "v"),
    ([8, 9], "v"),
    ([10, 11], "v"),
    ([12, 13], "v"),
    ([14], "g"),
    ([15], "g"),
]
# processing order (by expected data readiness given chain latencies)
PORD = [0, 2, 1, 5, 6, 3, 7, 4, 8, 9, 10]

_CACHE = {}


class _ForceExpIdentityTable:
    """Resolve Exp and Identity to the single table set containing both,
    so the ACT never reloads tables mid-kernel."""

    def __enter__(self):
        self._orig = bacc.get_activation_tables
        Exp = mybir.ActivationFunctionType.Exp
        Idn = mybir.ActivationFunctionType.Identity

        def patched(arch):
            tabs = self._orig(arch)
            out = {}
            for name, fns in tabs.items():
                if name != "exp_and_others" and (Exp in fns or Idn in fns):
                    fns = fns - {Exp, Idn}
                out[name] = fns
            return out

        bacc.get_activation_tables = patched
        return self

    def __exit__(self, *exc):
        bacc.get_activation_tables = self._orig


def _build():
    nc = bacc.Bacc(trn_type="TRN2")
    P = 128
    NG = len(GSPEC)

    xt_d = nc.dram_tensor("xt", [P, IN_BLOCKS, 2, BATCH], F16, kind="ExternalInput")
    wt_d = nc.dram_tensor("wt", [P, IN_BLOCKS, 2, IK], F16, kind="ExternalInput")
    y_d = nc.dram_tensor("y", [BATCH, IN_BLOCKS, IK], F16, kind="ExternalOutput")

    flat = "p a b k -> p (a b k)"
    MUL = mybir.AluOpType.mult
    ADD = mybir.AluOpType.add
    MAX = mybir.AluOpType.max

    with TileContext(nc) as tc:
        with (
            tc.tile_pool(name="singles", bufs=1) as singles,
            tc.tile_pool(name="wraw", bufs=NG) as wpool,
            tc.tile_pool(name="tv16", bufs=6) as vpool,
            tc.tile_pool(name="tg32", bufs=6) as gpool,
            tc.tile_pool(name="mtile", bufs=4) as mpool,
            tc.tile_pool(name="texp", bufs=4) as epool,
            tc.tile_pool(name="sab", bufs=4) as spool,
            tc.tile_pool(name="yout", bufs=3) as ypool,
            tc.tile_pool(name="yps", bufs=6, space="PSUM") as yps,
            tc.tile_pool(name="sps", bufs=1, space="PSUM") as sps,
        ):
            wt_t = [None] * NG
            m_t = [None] * NG
            e_t = [None] * NG
            sab_t = [None] * NG
            y_t = [None] * NG
            p_t = [None] * IN_BLOCKS

            def dma_w(g):
                js, _ = GSPEC[g]
                nj = len(js)
                wt_t[g] = wpool.tile([P, nj, 2, IK], F16, name="wraw", tag="wraw")
                nc.sync.dma_start(out=wt_t[g][:], in_=wt_d[:, js[0] : js[0] + nj])

            def chain(g):
                js, eng = GSPEC[g]
                fd = len(js) * 2 * IK
                w = wt_t[g][:].rearrange(flat)
                if eng == "v":
                    gg = vpool.tile([P, fd], F16, name="gg16", tag="tv16")
                    nc.vector.tensor_scalar(gg[:], w, A2M, P1, MUL, ADD)
                    hh = vpool.tile([P, fd], F16, name="hh16", tag="tv16")
                    nc.vector.tensor_tensor(hh[:], w, gg[:], MUL)
                    m_t[g] = mpool.tile([P, fd], F16, name="m16", tag="mtile")
                    nc.vector.tensor_scalar(
                        m_t[g][:], hh[:], P0 + MAGIC, MAGIC, ADD, MAX
                    )
                else:
                    gg = gpool.tile([P, fd], F32, name="gg32", tag="tg32")
                    nc.gpsimd.tensor_scalar(gg[:], w, A2M, P1, MUL, ADD)
                    hh = gpool.tile([P, fd], F32, name="hh32", tag="tg32")
                    nc.gpsimd.tensor_tensor(hh[:], w, gg[:], MUL)
                    m_t[g] = mpool.tile([P, fd], F16, name="m16", tag="mtile")
                    nc.gpsimd.tensor_scalar(
                        m_t[g][:], hh[:], P0 + MAGIC, MAGIC, ADD, MAX
                    )

            def exp_stage(g):
                js, _ = GSPEC[g]
                nj = len(js)
                e_t[g] = epool.tile([P, nj, 2, IK], F16, name="texp", tag="texp")
                nc.scalar.activation(
                    e_t[g][:].rearrange(flat),
                    m_t[g][:],
                    mybir.ActivationFunctionType.Exp,
                    bias=ebias[:, 0:1],
                    scale=LN_R,
                )

            def mm_stage(g):
                js, _ = GSPEC[g]
                for jrel, j in enumerate(js):
                    p_t[j] = yps.tile([P, IK], F32, name="yp", tag="yp")
                    for h in range(2):
                        nc.tensor.matmul(
                            s2_ps[:, j : j + 1],
                            xt_sb[:, j, h, :],
                            negones_sb[:],
                            start=(h == 0),
                            stop=(h == 1),
                        )
                        nc.tensor.matmul(
                            p_t[j][:],
                            xt_sb[:, j, h, :],
                            e_t[g][:, jrel, h, :],
                            start=(h == 0),
                            stop=(h == 1),
                        )

            def evict_stage(g):
                js, eng = GSPEC[g]
                nj = len(js)
                y_t[g] = ypool.tile([P, nj, IK], F16, name="ysb", tag="ysb")
                # s2_ps holds s2x = -s2 (ones preset to -1.0, exact in f16);
                # -A/B itself is NOT f16-representable, so scale here in f32
                if eng == "v":
                    # sa = -A/B*s2, then y = (p + sa)*(-B)
                    sab_t[g] = spool.tile([P, nj], F32, name="sab", tag="sab")
                    nc.vector.tensor_scalar(
                        sab_t[g][:],
                        s2_ps[:, js[0] : js[0] + nj],
                        A_OFF / B_SCALE,
                        None,
                        MUL,
                    )
                    for jrel, j in enumerate(js):
                        nc.vector.tensor_scalar(
                            y_t[g][:, jrel, :],
                            p_t[j][:],
                            sab_t[g][:, jrel : jrel + 1],
                            -B_SCALE,
                            ADD,
                            MUL,
                        )
                else:
                    # sab = A*s2 (SBUF), then ACT: y = -B*p + sab
                    sab_t[g] = spool.tile([P, nj], F32, name="sab", tag="sab")
                    nc.vector.tensor_scalar(
                        sab_t[g][:],
                        s2_ps[:, js[0] : js[0] + nj],
                        -A_OFF,
                        None,
                        MUL,
                    )
                    for jrel, j in enumerate(js):
                        nc.scalar.activation(
                            y_t[g][:, jrel, :],
                            p_t[j][:],
                            mybir.ActivationFunctionType.Identity,
                            bias=sab_t[g][:, jrel : jrel + 1],
                            scale=-B_SCALE,
                        )

            def store_stage(g):
                js, _ = GSPEC[g]
                nc.sync.dma_start(
                    out=y_d[:, js[0] : js[0] + len(js)], in_=y_t[g][:]
                )

            # --- prologue ---
            ebias = singles.tile([P, 1], F32)
            nc.vector.memset(ebias[:], EXP_BIAS)
            # tiny Exp forces the ACT table load before real work
            warm = singles.tile([P, 1], F32)
            nc.scalar.activation(
                warm[:], warm[:], mybir.ActivationFunctionType.Exp,
                bias=0.0, scale=0.0,
            )
            negones_sb = singles.tile([P, 1], F16)
            nc.vector.memset(negones_sb[:], -1.0)
            s2_ps = sps.tile([P, IN_BLOCKS], F32)
            # PE warm-up raises the HAM clock gate to 2.4 GHz
            warm_l = singles.tile([P, 16], F16)
            nc.vector.memset(warm_l[:], 0.5)
            warm_r = singles.tile([P, IK], F16)
            nc.vector.memset(warm_r[:], 0.5)
            wm_ps = sps.tile([16, IK], F32)
            for _ in range(8):
                nc.tensor.matmul(
                    wm_ps[:], warm_l[:], warm_r[:], start=True, stop=True
                )

            # all input DMAs issued upfront: weights j-ordered on the sync
            # ring, x in two chunks on the gpsimd ring
            xt_sb = singles.tile([P, IN_BLOCKS, 2, BATCH], F16)
            nc.gpsimd.dma_start(out=xt_sb[:, 0:8], in_=xt_d[:, 0:8])
            nc.gpsimd.dma_start(out=xt_sb[:, 8:16], in_=xt_d[:, 8:16])
            for g in range(NG):
                dma_w(g)

            # --- pipelined main loop in readiness order ---
            chain(PORD[0])
            chain(PORD[1])
            for idx, g in enumerate(PORD):
                if idx + 2 < NG:
                    chain(PORD[idx + 2])
                exp_stage(g)
                mm_stage(g)
                if idx >= 1:
                    evict_stage(PORD[idx - 1])
                    store_stage(PORD[idx - 1])
            evict_stage(PORD[-1])
            store_stage(PORD[-1])

    with _ForceExpIdentityTable():
        nc.compile()
    return nc


def _get_nc():
    if "nc" not in _CACHE:
        _CACHE["nc"] = _build()
    return _CACHE["nc"]


def kernel(x: np.ndarray, cores: np.ndarray, _trace=False, _trace_kwargs=None):
    x = np.asarray(x, dtype=np.float32)
    cores = np.asarray(cores, dtype=np.float32)

    xt = np.ascontiguousarray(
        x.T.reshape(IN_BLOCKS, 2, 128, BATCH)
        .transpose(2, 0, 1, 3)
        .astype(np.float16)
    )
    wt_full = (
        cores.reshape(OUT_BLOCKS, IN_BLOCKS, NB, 2, 128)  # i, j, k, h, p
        .transpose(4, 1, 3, 0, 2)  # p, j, h, i, k
        .astype(np.float16)
    )

    in_maps = []
    for c in range(N_CORES):
        wc = np.ascontiguousarray(
            wt_full[:, :, :, c * I_PER_CORE : (c + 1) * I_PER_CORE, :]
        ).reshape(128, IN_BLOCKS, 2, IK)
        in_maps.append({"xt": xt, "wt": wc})

    nc = _get_nc()
    kw = {}
    if _trace:
        kw = dict(trace=True, **(_trace_kwargs or {}))
    out = run_bass_kernel_spmd(nc, in_maps, core_ids=list(range(N_CORES)), **kw)
    if _trace:
        _CACHE["last_result"] = out
    y = np.concatenate(
        [
            r["y"]  # (b, j, (i,k))
            .astype(np.float32)
            .reshape(BATCH, IN_BLOCKS, I_PER_CORE, NB)
            .transpose(0, 2, 1, 3)
            for r in out.results
        ],
        axis=1,
    )
    return y
